# revision 34
# baseline (speedup 1.0000x reference)
"""Trainium2 Bass kernel for nn_BreakthroughSNN (spiking SSM LM).

v3 strategy (8 NeuronCores, SPMD single NEFF):
  - Data-parallel SSM: 2048 tokens (B*S) sharded 256/core; 4 layers x 20
    steps of LIF recurrence with membrane potentials resident in PSUM.
  - All SSM matmul operands in fp16: weights as host-split fp16 hi/lo
    pairs (11+11 mantissa bits ~ exact fp32 grade), spikes are {0,1}
    (exact in fp16).  Halves LDWEIGHTS traffic vs fp32r and avoids the
    fp32r producer-rounding rule.
  - Temporal encoding pipelined under layer 0: cumulative ge-planes
    (emb >= T_t) on DVE, one-hot diff on Pool, no separate encode phase.
  - LIF work spread so no engine exceeds the PE: ACT does Sign/masks,
    Pool does spike remaps, DVE only the PSUM read-modify-writes.
  - tips (time integration) matmuls emitted with 2-step skew so the PE
    never waits on the spike-write chain.
  - AllGather of bf16 rates; vocab-sharded projection transposed (vocab
    on PSUM partitions, per-partition bias, bf16 output).
"""

import numpy as np
import ml_dtypes
from contextlib import ExitStack

import concourse.bass as bass
import concourse.mybir as mybir
import concourse.tile as tile
from concourse import bacc
from concourse.bass_utils import run_bass_kernel_spmd
from concourse.masks import make_identity

F32 = mybir.dt.float32
F16 = mybir.dt.float16
BF16 = mybir.dt.bfloat16
I32 = mybir.dt.int32
OP = mybir.AluOpType
ACTF = mybir.ActivationFunctionType

NCORES = 8
TOKPC = 256          # tokens per core
BATCH, SEQ = 4, 512
DM, DS = 512, 128
T, L = 20, 4
VOC = 32000
VSH = VOC // NCORES  # 4000 vocab per core
KC = DM // 128       # 4 feature chunks
VB = 128             # vocab rows per projection block (full PE width)
NVBF = VOC // VB     # 250 blocks over the FULL vocab (per core)
SBJ = 5              # blocks per streamed super-block
SBV = VB * SBJ       # 640 vocab rows per super-block
NSB = NVBF // SBJ    # 50 streamed super-blocks
PREF = 16            # super-blocks prefetched during the SSM phase


def _hilo16(x):
    x = np.ascontiguousarray(x, dtype=np.float32)
    hi = x.astype(np.float16)
    lo = (x - hi.astype(np.float32)).astype(np.float16)
    return hi, lo


def _f2key(x):
    u = int(np.array(x, dtype=np.float32).view(np.uint32))
    return (u ^ 0x80000000) if u < 0x80000000 else (0xFFFFFFFF - u)


def _key2f(k):
    u = (k ^ 0x80000000) if k >= 0x80000000 else (0xFFFFFFFF - k)
    return np.array([u], dtype=np.uint32).view(np.float32)[0]


def _g32(x):
    # replicate reference fp32 pipeline: floor happens on this value
    x = np.float32(x)
    s = np.float32(1.0) / (np.float32(1.0) + np.float32(np.exp(np.float32(-x))))
    return np.float32(s * np.float32(19.0))


def _thresholds():
    """T_k = smallest fp32 x with g32(x) >= k, k=1..19 (g32 monotone)."""
    ts = []
    for k in range(1, 20):
        lo_k = _f2key(np.float32(-30.0))
        hi_k = _f2key(np.float32(30.0))
        assert _g32(_key2f(hi_k)) >= k and _g32(_key2f(lo_k)) < k
        while hi_k - lo_k > 1:
            mid = (lo_k + hi_k) // 2
            if _g32(_key2f(mid)) >= k:
                hi_k = mid
            else:
                lo_k = mid
        ts.append(float(_key2f(hi_k)))
    return ts


def _build_nc():
    nc = bacc.Bacc("TRN2", target_bir_lowering=False, debug=False, num_devices=NCORES)

    ids_d = nc.dram_tensor("ids", [2, 128, 1], I32, kind="ExternalInput")
    emb_d = nc.dram_tensor("emb", [VOC, DM], F32, kind="ExternalInput")
    at_hi_d = nc.dram_tensor("at_hi", [L, 128, 128], F16, kind="ExternalInput")
    at_lo_d = nc.dram_tensor("at_lo", [L, 128, 128], F16, kind="ExternalInput")
    bt_hi_d = nc.dram_tensor("bt_hi", [L, 128, KC, 128], F16, kind="ExternalInput")
    bt_lo_d = nc.dram_tensor("bt_lo", [L, 128, KC, 128], F16, kind="ExternalInput")
    ct_hi_d = nc.dram_tensor("ct_hi", [L, 128, KC, 128], F16, kind="ExternalInput")
    ct_lo_d = nc.dram_tensor("ct_lo", [L, 128, KC, 128], F16, kind="ExternalInput")
    # D hi/lo values (fp16-representable) stored as f32 for the scalar AP
    dc_hi_d = nc.dram_tensor("dc_hi", [128, L * KC], F32, kind="ExternalInput")
    dc_lo_d = nc.dram_tensor("dc_lo", [128, L * KC], F32, kind="ExternalInput")
    wptb_d = nc.dram_tensor("wptb", [NSB, 128, KC, SBV], BF16, kind="ExternalInput")
    bias_d = nc.dram_tensor("bias", [VB, NVBF], F32, kind="ExternalInput")
    out_d = nc.dram_tensor("out", [VOC, TOKPC], BF16, kind="ExternalOutput")

    THR = _thresholds()

    with tile.TileContext(nc) as tc, ExitStack() as ctx:
        const = ctx.enter_context(tc.tile_pool(name="const", bufs=1))
        ident = const.tile([128, 128], F32)
        make_identity(nc, ident[:])
        ident16 = const.tile([128, 128], F16)
        nc.gpsimd.tensor_scalar(ident16[:], ident[:], 1.0, 0.0, OP.mult, OP.add)
        neg2 = const.tile([128, 1], F32)
        nc.vector.memset(neg2[:], -2.0)

        # ---- persistent SBUF state ----
        xb_pool = ctx.enter_context(tc.tile_pool(name="xb", bufs=1))
        xb = xb_pool.tile([128, T * KC * 256], F16)
        tip = ctx.enter_context(tc.tile_pool(name="ti", bufs=1))
        tibf = tip.tile([128, KC * 256], BF16, tag="tibf")
        embp = ctx.enter_context(tc.tile_pool(name="embp", bufs=1))
        EMBall = embp.tile([128, KC * 256], F32, tag="emba")
        gep = ctx.enter_context(tc.tile_pool(name="gep", bufs=3))

        # ---- encode inputs first: gather + transpose (short critical path) --
        with tc.tile_pool(name="enc", bufs=2) as enc, \
             tc.tile_pool(name="encp", bufs=2, space="PSUM") as encps:
            ids_s = enc.tile([128, 2], I32, tag="ids")
            for g in range(2):
                nc.sync.dma_start(ids_s[:, g:g + 1], ids_d[g, :, :])
            for g in range(2):
                eg = enc.tile([128, DM], F32, tag="eg")
                nc.gpsimd.indirect_dma_start(
                    out=eg[:], out_offset=None,
                    in_=emb_d[:, :],
                    in_offset=bass.IndirectOffsetOnAxis(ap=ids_s[:, g:g + 1], axis=0),
                )
                for k in range(KC):
                    pt = encps.tile([128, 128], F32, tag="pt")
                    nc.tensor.transpose(pt[:], eg[:, k * 128:(k + 1) * 128], ident[:])
                    nc.scalar.copy(EMBall[:, k * 256 + g * 128:k * 256 + g * 128 + 128],
                                   pt[:])

        # ge-plane pipeline: ge[t] = (emb >= T_t), xb[t] = ge[t] - ge[t+1]
        ges = {}

        def make_ge(t):
            # t in 1..19; ge_20 == 0, ge_0 == 1  ({0,1} exact in fp16)
            g_ = gep.tile([128, KC * 256], F16, tag="ge", name=f"ge{t}", bufs=4)
            nc.vector.tensor_scalar(g_[:], EMBall[:], float(THR[t - 1]), None,
                                    OP.is_ge)
            ges[t] = g_

        def make_onehot(t):
            # Pool diff into fp16 xb plane
            dst = xb[:, t * KC * 256:(t + 1) * KC * 256]
            if t == 0:
                # 1 - ge_1
                nc.gpsimd.tensor_scalar(dst, ges[1][:], -1.0, 1.0, OP.mult, OP.add)
            elif t == T - 1:
                nc.gpsimd.tensor_scalar(dst, ges[T - 1][:], 1.0, 0.0, OP.mult, OP.add)
            else:
                nc.gpsimd.tensor_tensor(dst, ges[t][:], ges[t + 1][:],
                                        OP.subtract)

        make_ge(1)
        make_onehot(0)
        make_ge(2)
        make_onehot(1)
        make_ge(3)
        make_onehot(2)

        # ---- all-layer parameters (fp16, plain DMA) ----
        par = ctx.enter_context(tc.tile_pool(name="par", bufs=1))
        AH, AL, BH, BL, CH, CL = [], [], [], [], [], []
        for l in range(L):
            for nm, lst, dram, shp in (
                    ("ah", AH, at_hi_d, (128, 128)), ("al", AL, at_lo_d, (128, 128)),
                    ("bh", BH, bt_hi_d, (128, KC, 128)), ("bl", BL, bt_lo_d, (128, KC, 128)),
                    ("ch", CH, ct_hi_d, (128, KC, 128)), ("cl", CL, ct_lo_d, (128, KC, 128))):
                t_ = par.tile(list(shp), F16, tag=f"p_{nm}_{l}", name=f"par{l}{nm}")
                nc.sync.dma_start(t_[:], dram[l])
                lst.append(t_)
        dch_all = par.tile([128, L * KC], F32, tag="dch")
        nc.sync.dma_start(dch_all[:], dc_hi_d[:, :])
        dcl_all = par.tile([128, L * KC], F32, tag="dcl")
        nc.sync.dma_start(dcl_all[:], dc_lo_d[:, :])

        # ---- projection weights: streamed super-blocks; prefetch the first
        # PREF during the (HBM-idle) SSM phase via the Pool DMA queue ----
        wpp = ctx.enter_context(tc.tile_pool(name="wpp", bufs=PREF))
        wpb_tiles = {}

        def fetch_wpb(sb):
            w = wpp.tile([128, KC, SBV], BF16, tag="wpb", name=f"wpb{sb}",
                         bufs=PREF)
            nc.sync.dma_start(w[:], wptb_d[sb])
            wpb_tiles[sb] = w

        biasp = ctx.enter_context(tc.tile_pool(name="biasp", bufs=1))
        biasb = biasp.tile([VB, NVBF], F32, tag="biasb")
        nc.sync.dma_start(biasb[:], bias_d[:, :])
        for sb in range(PREF):
            fetch_wpb(sb)

        # ---------------- SSM layers ---------------------------------------
        with tc.tile_pool(name="ssmp", bufs=1, space="PSUM") as ssmps, \
             tc.tile_pool(name="dgp", bufs=2) as dgp, \
             tc.tile_pool(name="lif", bufs=3) as lif:
            v1ps = ssmps.tile([128, TOKPC], F32, tag="v1")
            v2pr = [ssmps.tile([128, 2 * TOKPC], F32, tag=f"v2p{j}", name=f"v2pr{j}")
                    for j in range(2)]
            tips = ssmps.tile([128, KC * TOKPC], F32, tag="tips")

            Hprev = None
            tips_pend = []  # deferred tips matmuls (2-step skew)
            for layer in range(L):
                # diagonal D tiles for this layer, built on Pool (fast path)
                ddh, ddl = [], []
                for k in range(KC):
                    dt_ = dgp.tile([128, 128], F16, tag=f"ddh{k}", name=f"ddh{k}")
                    nc.gpsimd.tensor_scalar(
                        dt_[:], ident[:],
                        dch_all[:, layer * KC + k:layer * KC + k + 1], 0.0,
                        OP.mult, OP.add)
                    ddh.append(dt_)
                    dt_ = dgp.tile([128, 128], F16, tag=f"ddl{k}", name=f"ddl{k}")
                    nc.gpsimd.tensor_scalar(
                        dt_[:], ident[:],
                        dcl_all[:, layer * KC + k:layer * KC + k + 1], 0.0,
                        OP.mult, OP.add)
                    ddl.append(dt_)
                ahT, alT = AH[layer], AL[layer]
                bhT, blT = BH[layer], BL[layer]
                chT, clT = CH[layer], CL[layer]

                def flush_tips(keep=0):
                    while len(tips_pend) > keep:
                        t_, k_, xs_ = tips_pend.pop(0)
                        nc.tensor.matmul(
                            tips[:, k_ * TOKPC:(k_ + 1) * TOKPC],
                            ident16[:], xs_,
                            start=(t_ == 0 and k_ % 2 == 0),
                            stop=(t_ == T - 1),
                            skip_group_check=True)

                def emit_mm2_lif2(t, H_t, xs_t, layer_):
                    # output update accumulation (v2, per chunk) + LIF2
                    for k in range(KC):
                        vsl = v2pr[k // 2][:, (k % 2) * TOKPC:(k % 2 + 1) * TOKPC]
                        mm2 = [(chT[:, k, :], H_t[:]), (clT[:, k, :], H_t[:]),
                               (ddh[k][:], xs_t[k]), (ddl[k][:], xs_t[k])]
                        for i, (lhsT, rhs) in enumerate(mm2):
                            nc.tensor.matmul(vsl, lhsT, rhs,
                                             start=(t == 0 and i == 0 and k % 2 == 0),
                                             stop=(i == len(mm2) - 1),
                                             skip_group_check=True)
                    # j=0: DVE mask from PSUM, Pool spike remap
                    m2 = lif.tile([128, 2 * TOKPC], F32, tag="m2_0", name="m2_0")
                    nc.vector.tensor_scalar(m2[:], v2pr[0][:], 2.0, 0.5,
                                            OP.is_lt, OP.mult)
                    nc.vector.tensor_tensor(v2pr[0][:], v2pr[0][:], m2[:], OP.mult)
                    xsl0 = xb[:, t * KC * 256:t * KC * 256 + 512]
                    nc.gpsimd.tensor_scalar(xsl0, m2[:], -2.0, 1.0, OP.mult, OP.add)
                    # j=1: ACT sign + mask, Pool spike remap
                    sg2 = lif.tile([128, 2 * TOKPC], F32, tag="sg2_1", name="sg2_1")
                    nc.scalar.activation(sg2[:], v2pr[1][:], ACTF.Sign,
                                         bias=neg2[:], scale=1.0)
                    m2b = lif.tile([128, 2 * TOKPC], F32, tag="m2_1", name="m2_1")
                    nc.scalar.activation(m2b[:], sg2[:], ACTF.Copy,
                                         bias=0.25, scale=-0.25)
                    nc.vector.tensor_tensor(v2pr[1][:], v2pr[1][:], m2b[:], OP.mult)
                    xsl1 = xb[:, t * KC * 256 + 512:t * KC * 256 + 1024]
                    nc.gpsimd.tensor_scalar(xsl1, sg2[:], 0.5, 0.5, OP.mult, OP.add)
                    if layer_ == L - 1:
                        for k in range(KC):
                            tips_pend.append((t, k, xs_t[k]))

                prev = None  # (t, H, xs) pending MM2+LIF2 (1-step software skew)
                for t in range(T):
                    xs = [xb[:, (t * KC + k) * 256:(t * KC + k) * 256 + 256]
                          for k in range(KC)]
                    # ---- state update accumulation (v1): B first, A last ----
                    mm1 = []
                    for k in range(KC):
                        mm1 += [(bhT[:, k, :], xs[k]), (blT[:, k, :], xs[k])]
                    if t > 0:
                        mm1 += [(ahT[:], Hprev[:]), (alT[:], Hprev[:])]
                    for i, (lhsT, rhs) in enumerate(mm1):
                        nc.tensor.matmul(v1ps[:], lhsT, rhs,
                                         start=(t == 0 and i == 0),
                                         stop=(i == len(mm1) - 1),
                                         skip_group_check=True)
                    # ---- LIF1: ACT sign, Pool masks, DVE RMW only ----
                    sg1 = lif.tile([128, TOKPC], F32, tag="sg1")
                    nc.scalar.activation(sg1[:], v1ps[:], ACTF.Sign,
                                         bias=neg2[:], scale=1.0)
                    m1 = lif.tile([128, TOKPC], F32, tag="m1")
                    nc.gpsimd.tensor_scalar(m1[:], sg1[:], -0.25, 0.25,
                                            OP.mult, OP.add)
                    nc.vector.tensor_tensor(v1ps[:], v1ps[:], m1[:], OP.mult)
                    H = lif.tile([128, TOKPC], F16, tag="H", bufs=3)
                    nc.gpsimd.tensor_scalar(H[:], sg1[:], 0.5, 0.5,
                                            OP.mult, OP.add)
                    # encode pipeline: prepare one-hot planes ahead (layer 0)
                    if layer == 0:
                        if t + 4 < T:
                            make_ge(t + 4)
                        if t + 3 < T:
                            make_onehot(t + 3)
                    # ---- previous step's output-side work (keeps PE fed) ----
                    if prev is not None:
                        emit_mm2_lif2(*prev, layer)
                        if layer == L - 1:
                            flush_tips(keep=4)
                    prev = (t, H, xs)
                    Hprev = H
                emit_mm2_lif2(*prev, layer)
                flush_tips()

            # time-integrated rates -> bf16 (ACT, scale = 1/T)
            for k in range(KC):
                nc.scalar.activation(tibf[:, k * 256:(k + 1) * 256],
                                     tips[:, k * TOKPC:(k + 1) * TOKPC],
                                     ACTF.Copy, bias=0.0, scale=1.0 / T)

        # ---- full-vocab projection of this core's own 256 tokens ----------
        # (no collective: Wp is streamed in full, out is token-sharded)
        with tc.tile_pool(name="prjp", bufs=6, space="PSUM") as prjps, \
             tc.tile_pool(name="osb", bufs=6) as osbp:
            for sb in range(NSB):
                if sb + PREF < NSB:
                    fetch_wpb(sb + PREF)
                wpb = wpb_tiles[sb]
                for j in range(SBJ):
                    vb = sb * SBJ + j
                    po = prjps.tile([VB, TOKPC], F32, tag="po", name="po",
                                    bufs=6)
                    for k in range(KC):
                        nc.tensor.matmul(po[:],
                                         wpb[:, k, j * VB:(j + 1) * VB],
                                         tibf[:, k * 256:(k + 1) * 256],
                                         start=(k == 0), stop=(k == KC - 1),
                                         skip_group_check=True)
                    osb = osbp.tile([VB, TOKPC], BF16, tag="osb")
                    nc.vector.tensor_scalar(osb[:], po[:],
                                            biasb[:, vb:vb + 1], None, OP.add)
                    eng = nc.sync if vb % 2 == 0 else nc.scalar
                    eng.dma_start(out_d[vb * VB:(vb + 1) * VB, :], osb[:])

    nc.compile()
    return nc


_NC_CACHE = {}
_last_in_maps = None


def _get_nc():
    if "nc" not in _NC_CACHE:
        _NC_CACHE["nc"] = _build_nc()
    return _NC_CACHE["nc"]


def kernel(input_ids, emb_table, A, B, C, D, Wp, bp):
    input_ids = np.asarray(input_ids)
    emb_table = np.ascontiguousarray(np.asarray(emb_table), dtype=np.float32)
    A = np.asarray(A, dtype=np.float32)
    B = np.asarray(B, dtype=np.float32)
    C = np.asarray(C, dtype=np.float32)
    D = np.asarray(D, dtype=np.float32)
    Wp = np.asarray(Wp, dtype=np.float32)
    bp = np.asarray(bp, dtype=np.float32)

    ids_flat = input_ids.reshape(-1).astype(np.int32)          # (2048,)

    at = np.ascontiguousarray(A.transpose(0, 2, 1))            # (L,128,128)
    at_hi, at_lo = _hilo16(at)
    bt = np.ascontiguousarray(
        B.transpose(2, 0, 1).reshape(KC, 128, L, DS).transpose(2, 1, 0, 3))
    # bt[l,p,k,m] = B[l, m, k*128+p]
    bt_hi, bt_lo = _hilo16(bt)
    ct = np.ascontiguousarray(C.transpose(0, 2, 1).reshape(L, 128, KC, 128))
    # ct[l,p,mc,m] = C[l, mc*128+m, p]
    ct_hi, ct_lo = _hilo16(ct)
    # dc[p, l*KC+k] = D[l, k*128+p]; fp16-rounded values shipped as f32
    dc = np.ascontiguousarray(
        D.reshape(L, KC, 128).transpose(2, 0, 1).reshape(128, L * KC))
    dc_hi16, dc_lo16 = _hilo16(dc)
    dc_hi = dc_hi16.astype(np.float32)
    dc_lo = dc_lo16.astype(np.float32)

    wpt = np.ascontiguousarray(Wp.T)                           # (512, 32000) f32
    wpt_bf = wpt.astype(ml_dtypes.bfloat16)
    # wptb[sb, p, k, v] = Wp.T[k*128+p, sb*640+v]  (full vocab, shared)
    wptb = np.ascontiguousarray(
        wpt_bf.reshape(KC, 128, NSB, SBV).transpose(2, 1, 0, 3))
    bsh = np.ascontiguousarray(bp.reshape(NVBF, VB).T)         # (128, 250)

    nc = _get_nc()
    in_maps = []
    for c in range(NCORES):
        ids_c = ids_flat[c * TOKPC:(c + 1) * TOKPC].reshape(2, 128, 1)
        in_maps.append({
            "ids": np.ascontiguousarray(ids_c),
            "emb": emb_table,
            "at_hi": at_hi, "at_lo": at_lo,
            "bt_hi": bt_hi, "bt_lo": bt_lo,
            "ct_hi": ct_hi, "ct_lo": ct_lo,
            "dc_hi": dc_hi, "dc_lo": dc_lo,
            "wptb": wptb,
            "bias": bsh,
        })

    global _last_in_maps
    _last_in_maps = in_maps
    res = run_bass_kernel_spmd(nc, in_maps, core_ids=list(range(NCORES)))
    outs = [np.asarray(res.results[c]["out"]) for c in range(NCORES)]
    full = np.concatenate(outs, axis=1)                        # (32000, 2048) bf16
    full = full.astype(np.float32).T                           # (2048, 32000)
    return np.ascontiguousarray(full).reshape(BATCH, SEQ, VOC)


# revision 39
# speedup vs baseline: 1.0904x; 1.0904x over previous
"""Trainium2 Bass kernel for nn_BreakthroughSNN (spiking SSM LM).

v3 strategy (8 NeuronCores, SPMD single NEFF):
  - Data-parallel SSM: 2048 tokens (B*S) sharded 256/core; 4 layers x 20
    steps of LIF recurrence with membrane potentials resident in PSUM.
  - All SSM matmul operands in fp16: weights as host-split fp16 hi/lo
    pairs (11+11 mantissa bits ~ exact fp32 grade), spikes are {0,1}
    (exact in fp16).  Halves LDWEIGHTS traffic vs fp32r and avoids the
    fp32r producer-rounding rule.
  - Temporal encoding pipelined under layer 0: cumulative ge-planes
    (emb >= T_t) on DVE, one-hot diff on Pool, no separate encode phase.
  - LIF work spread so no engine exceeds the PE: ACT does Sign/masks,
    Pool does spike remaps, DVE only the PSUM read-modify-writes.
  - tips (time integration) matmuls emitted with 2-step skew so the PE
    never waits on the spike-write chain.
  - AllGather of bf16 rates; vocab-sharded projection transposed (vocab
    on PSUM partitions, per-partition bias, bf16 output).
"""

import numpy as np
import ml_dtypes
from contextlib import ExitStack

import concourse.bass as bass
import concourse.mybir as mybir
import concourse.tile as tile
from concourse import bacc
from concourse.bass_utils import run_bass_kernel_spmd
from concourse.masks import make_identity

F32 = mybir.dt.float32
F16 = mybir.dt.float16
BF16 = mybir.dt.bfloat16
I32 = mybir.dt.int32
OP = mybir.AluOpType
ACTF = mybir.ActivationFunctionType

NCORES = 8
TOKPC = 256          # tokens per core
BATCH, SEQ = 4, 512
DM, DS = 512, 128
T, L = 20, 4
VOC = 32000
VSH = VOC // NCORES  # 4000 vocab per core
KC = DM // 128       # 4 feature chunks
VB = 128             # vocab rows per projection block (full PE width)
NVBF = VOC // VB     # 250 blocks over the FULL vocab (per core)
SBJ = 5              # blocks per streamed super-block
SBV = VB * SBJ       # 640 vocab rows per super-block
NSB = NVBF // SBJ    # 50 streamed super-blocks
PREF = 16            # super-blocks prefetched during the SSM phase


def _hilo16(x):
    x = np.ascontiguousarray(x, dtype=np.float32)
    hi = x.astype(np.float16)
    lo = (x - hi.astype(np.float32)).astype(np.float16)
    return hi, lo


def _f2key(x):
    u = int(np.array(x, dtype=np.float32).view(np.uint32))
    return (u ^ 0x80000000) if u < 0x80000000 else (0xFFFFFFFF - u)


def _key2f(k):
    u = (k ^ 0x80000000) if k >= 0x80000000 else (0xFFFFFFFF - k)
    return np.array([u], dtype=np.uint32).view(np.float32)[0]


def _g32(x):
    # replicate reference fp32 pipeline: floor happens on this value
    x = np.float32(x)
    s = np.float32(1.0) / (np.float32(1.0) + np.float32(np.exp(np.float32(-x))))
    return np.float32(s * np.float32(19.0))


def _thresholds():
    """T_k = smallest fp32 x with g32(x) >= k, k=1..19 (g32 monotone)."""
    ts = []
    for k in range(1, 20):
        lo_k = _f2key(np.float32(-30.0))
        hi_k = _f2key(np.float32(30.0))
        assert _g32(_key2f(hi_k)) >= k and _g32(_key2f(lo_k)) < k
        while hi_k - lo_k > 1:
            mid = (lo_k + hi_k) // 2
            if _g32(_key2f(mid)) >= k:
                hi_k = mid
            else:
                lo_k = mid
        ts.append(float(_key2f(hi_k)))
    return ts


def _build_nc():
    nc = bacc.Bacc("TRN2", target_bir_lowering=False, debug=False, num_devices=NCORES)

    ids_d = nc.dram_tensor("ids", [2, 128, 1], I32, kind="ExternalInput")
    emb_d = nc.dram_tensor("emb", [VOC, DM], F32, kind="ExternalInput")
    at_hi_d = nc.dram_tensor("at_hi", [L, 128, 128], F16, kind="ExternalInput")
    at_lo_d = nc.dram_tensor("at_lo", [L, 128, 128], F16, kind="ExternalInput")
    bt_hi_d = nc.dram_tensor("bt_hi", [L, 128, KC, 128], F16, kind="ExternalInput")
    bt_lo_d = nc.dram_tensor("bt_lo", [L, 128, KC, 128], F16, kind="ExternalInput")
    ct_hi_d = nc.dram_tensor("ct_hi", [L, 128, KC, 128], F16, kind="ExternalInput")
    ct_lo_d = nc.dram_tensor("ct_lo", [L, 128, KC, 128], F16, kind="ExternalInput")
    # D hi/lo values (fp16-representable) stored as f32 for the scalar AP
    dc_hi_d = nc.dram_tensor("dc_hi", [128, L * KC], F32, kind="ExternalInput")
    dc_lo_d = nc.dram_tensor("dc_lo", [128, L * KC], F32, kind="ExternalInput")
    wptb_d = nc.dram_tensor("wptb", [NSB, 128, KC, SBV], BF16, kind="ExternalInput")
    bias_d = nc.dram_tensor("bias", [VB, NVBF], F32, kind="ExternalInput")
    out_d = nc.dram_tensor("out", [VOC, TOKPC], BF16, kind="ExternalOutput")

    THR = _thresholds()

    with tile.TileContext(nc) as tc, ExitStack() as ctx:
        const = ctx.enter_context(tc.tile_pool(name="const", bufs=1))
        ident = const.tile([128, 128], F32)
        make_identity(nc, ident[:])
        ident16 = const.tile([128, 128], F16)
        nc.gpsimd.tensor_scalar(ident16[:], ident[:], 1.0, 0.0, OP.mult, OP.add)
        neg2 = const.tile([128, 1], F32)
        nc.vector.memset(neg2[:], -2.0)

        # ---- persistent SBUF state ----
        xb_pool = ctx.enter_context(tc.tile_pool(name="xb", bufs=1))
        xb = xb_pool.tile([128, T * KC * 256], F16)
        tip = ctx.enter_context(tc.tile_pool(name="ti", bufs=1))
        tibf = tip.tile([128, KC * 256], BF16, tag="tibf")
        embp = ctx.enter_context(tc.tile_pool(name="embp", bufs=1))
        EMBall = embp.tile([128, KC * 256], F32, tag="emba")
        gep = ctx.enter_context(tc.tile_pool(name="gep", bufs=3))

        # ---- encode inputs first: gather + transpose (short critical path) --
        with tc.tile_pool(name="enc", bufs=2) as enc, \
             tc.tile_pool(name="encp", bufs=2, space="PSUM") as encps:
            ids_s = enc.tile([128, 2], I32, tag="ids")
            for g in range(2):
                nc.sync.dma_start(ids_s[:, g:g + 1], ids_d[g, :, :])
            for g in range(2):
                eg = enc.tile([128, DM], F32, tag="eg")
                nc.gpsimd.indirect_dma_start(
                    out=eg[:], out_offset=None,
                    in_=emb_d[:, :],
                    in_offset=bass.IndirectOffsetOnAxis(ap=ids_s[:, g:g + 1], axis=0),
                )
                for k in range(KC):
                    pt = encps.tile([128, 128], F32, tag="pt")
                    nc.tensor.transpose(pt[:], eg[:, k * 128:(k + 1) * 128], ident[:])
                    nc.scalar.copy(EMBall[:, k * 256 + g * 128:k * 256 + g * 128 + 128],
                                   pt[:])

        # ge-plane pipeline: ge[t] = (emb >= T_t), xb[t] = ge[t] - ge[t+1]
        ges = {}

        def make_ge(t):
            # t in 1..19; ge_20 == 0, ge_0 == 1  ({0,1} exact in fp16)
            g_ = gep.tile([128, KC * 256], F16, tag="ge", name=f"ge{t}", bufs=4)
            nc.vector.tensor_scalar(g_[:], EMBall[:], float(THR[t - 1]), None,
                                    OP.is_ge)
            ges[t] = g_

        def make_onehot(t):
            # diff into fp16 xb plane; alternate DVE/Pool (Pool TT is slow,
            # DVE gets 2x throughput on 16-bit)
            dst = xb[:, t * KC * 256:(t + 1) * KC * 256]
            eng = nc.vector if t % 2 == 0 else nc.gpsimd
            if t == 0:
                # 1 - ge_1
                eng.tensor_scalar(dst, ges[1][:], -1.0, 1.0, OP.mult, OP.add)
            elif t == T - 1:
                eng.tensor_scalar(dst, ges[T - 1][:], 1.0, 0.0, OP.mult, OP.add)
            else:
                eng.tensor_tensor(dst, ges[t][:], ges[t + 1][:], OP.subtract)

        make_ge(1)
        make_onehot(0)
        make_ge(2)
        make_onehot(1)
        make_ge(3)
        make_onehot(2)

        # ---- all-layer parameters (fp16, plain DMA) ----
        par = ctx.enter_context(tc.tile_pool(name="par", bufs=1))
        AH, AL, BH, BL, CH, CL = [], [], [], [], [], []
        for l in range(L):
            for nm, lst, dram, shp in (
                    ("ah", AH, at_hi_d, (128, 128)), ("al", AL, at_lo_d, (128, 128)),
                    ("bh", BH, bt_hi_d, (128, KC, 128)), ("bl", BL, bt_lo_d, (128, KC, 128)),
                    ("ch", CH, ct_hi_d, (128, KC, 128)), ("cl", CL, ct_lo_d, (128, KC, 128))):
                t_ = par.tile(list(shp), F16, tag=f"p_{nm}_{l}", name=f"par{l}{nm}")
                nc.sync.dma_start(t_[:], dram[l])
                lst.append(t_)
        dch_all = par.tile([128, L * KC], F32, tag="dch")
        nc.sync.dma_start(dch_all[:], dc_hi_d[:, :])
        dcl_all = par.tile([128, L * KC], F32, tag="dcl")
        nc.sync.dma_start(dcl_all[:], dc_lo_d[:, :])

        # ---- projection weights: streamed super-blocks; prefetch the first
        # PREF during the (HBM-idle) SSM phase via the Pool DMA queue ----
        wpp = ctx.enter_context(tc.tile_pool(name="wpp", bufs=PREF))
        wpb_tiles = {}

        def fetch_wpb(sb):
            w = wpp.tile([128, KC, SBV], BF16, tag="wpb", name=f"wpb{sb}",
                         bufs=PREF)
            # upfront prefetch rides the sync queue (idle during SSM); the
            # in-projection stream rides the Pool queue (idle then) so
            # neither blocks behind output DMAs
            eng = nc.sync if sb < PREF else nc.gpsimd
            eng.dma_start(w[:], wptb_d[sb])
            wpb_tiles[sb] = w

        biasp = ctx.enter_context(tc.tile_pool(name="biasp", bufs=1))
        biasb = biasp.tile([VB, NVBF], F32, tag="biasb")
        nc.sync.dma_start(biasb[:], bias_d[:, :])
        for sb in range(PREF):
            fetch_wpb(sb)

        # ---------------- SSM layers ---------------------------------------
        with tc.tile_pool(name="ssmp", bufs=1, space="PSUM") as ssmps, \
             tc.tile_pool(name="dgp", bufs=2) as dgp, \
             tc.tile_pool(name="lif", bufs=3) as lif:
            v1ps = ssmps.tile([128, TOKPC], F32, tag="v1")
            v2pr = [ssmps.tile([128, 2 * TOKPC], F32, tag=f"v2p{j}", name=f"v2pr{j}")
                    for j in range(2)]
            tips = ssmps.tile([128, KC * TOKPC], F32, tag="tips")

            Hprev = None
            tips_pend = []  # deferred tips matmuls (2-step skew)
            for layer in range(L):
                # diagonal D tiles for this layer, built on Pool (fast path)
                ddh, ddl = [], []
                for k in range(KC):
                    dt_ = dgp.tile([128, 128], F16, tag=f"ddh{k}", name=f"ddh{k}")
                    nc.gpsimd.tensor_scalar(
                        dt_[:], ident[:],
                        dch_all[:, layer * KC + k:layer * KC + k + 1], 0.0,
                        OP.mult, OP.add)
                    ddh.append(dt_)
                    dt_ = dgp.tile([128, 128], F16, tag=f"ddl{k}", name=f"ddl{k}")
                    nc.gpsimd.tensor_scalar(
                        dt_[:], ident[:],
                        dcl_all[:, layer * KC + k:layer * KC + k + 1], 0.0,
                        OP.mult, OP.add)
                    ddl.append(dt_)
                ahT, alT = AH[layer], AL[layer]
                bhT, blT = BH[layer], BL[layer]
                chT, clT = CH[layer], CL[layer]

                def flush_tips(keep=0):
                    while len(tips_pend) > keep:
                        t_, k_, xs_ = tips_pend.pop(0)
                        nc.tensor.matmul(
                            tips[:, k_ * TOKPC:(k_ + 1) * TOKPC],
                            ident16[:], xs_,
                            start=(t_ == 0 and k_ % 2 == 0),
                            stop=(t_ == T - 1),
                            skip_group_check=True)

                def emit_mm2_lif2(t, H_t, xs_t, layer_):
                    # output update accumulation (v2, per chunk) + LIF2
                    for k in range(KC):
                        vsl = v2pr[k // 2][:, (k % 2) * TOKPC:(k % 2 + 1) * TOKPC]
                        mm2 = [(chT[:, k, :], H_t[:]), (clT[:, k, :], H_t[:]),
                               (ddh[k][:], xs_t[k]), (ddl[k][:], xs_t[k])]
                        for i, (lhsT, rhs) in enumerate(mm2):
                            nc.tensor.matmul(vsl, lhsT, rhs,
                                             start=(t == 0 and i == 0 and k % 2 == 0),
                                             stop=(i == len(mm2) - 1),
                                             skip_group_check=True)
                    # j=0: DVE mask from PSUM, Pool spike remap
                    m2 = lif.tile([128, 2 * TOKPC], F32, tag="m2_0", name="m2_0")
                    nc.vector.tensor_scalar(m2[:], v2pr[0][:], 2.0, 0.5,
                                            OP.is_lt, OP.mult)
                    nc.vector.tensor_tensor(v2pr[0][:], v2pr[0][:], m2[:], OP.mult)
                    xsl0 = xb[:, t * KC * 256:t * KC * 256 + 512]
                    nc.gpsimd.tensor_scalar(xsl0, m2[:], -2.0, 1.0, OP.mult, OP.add)
                    # j=1: ACT sign + mask, Pool spike remap
                    sg2 = lif.tile([128, 2 * TOKPC], F32, tag="sg2_1", name="sg2_1")
                    nc.scalar.activation(sg2[:], v2pr[1][:], ACTF.Sign,
                                         bias=neg2[:], scale=1.0)
                    m2b = lif.tile([128, 2 * TOKPC], F32, tag="m2_1", name="m2_1")
                    nc.scalar.activation(m2b[:], sg2[:], ACTF.Copy,
                                         bias=0.25, scale=-0.25)
                    nc.vector.tensor_tensor(v2pr[1][:], v2pr[1][:], m2b[:], OP.mult)
                    xsl1 = xb[:, t * KC * 256 + 512:t * KC * 256 + 1024]
                    nc.gpsimd.tensor_scalar(xsl1, sg2[:], 0.5, 0.5, OP.mult, OP.add)
                    if layer_ == L - 1:
                        for k in range(KC):
                            tips_pend.append((t, k, xs_t[k]))

                prev = None  # (t, H, xs) pending MM2+LIF2 (1-step software skew)
                for t in range(T):
                    xs = [xb[:, (t * KC + k) * 256:(t * KC + k) * 256 + 256]
                          for k in range(KC)]
                    # ---- state update accumulation (v1): B first, A last ----
                    mm1 = []
                    for k in range(KC):
                        mm1 += [(bhT[:, k, :], xs[k]), (blT[:, k, :], xs[k])]
                    if t > 0:
                        mm1 += [(ahT[:], Hprev[:]), (alT[:], Hprev[:])]
                    for i, (lhsT, rhs) in enumerate(mm1):
                        nc.tensor.matmul(v1ps[:], lhsT, rhs,
                                         start=(t == 0 and i == 0),
                                         stop=(i == len(mm1) - 1),
                                         skip_group_check=True)
                    # ---- LIF1: ACT sign, Pool masks, DVE RMW only ----
                    sg1 = lif.tile([128, TOKPC], F32, tag="sg1")
                    nc.scalar.activation(sg1[:], v1ps[:], ACTF.Sign,
                                         bias=neg2[:], scale=1.0)
                    m1 = lif.tile([128, TOKPC], F32, tag="m1")
                    nc.gpsimd.tensor_scalar(m1[:], sg1[:], -0.25, 0.25,
                                            OP.mult, OP.add)
                    nc.vector.tensor_tensor(v1ps[:], v1ps[:], m1[:], OP.mult)
                    H = lif.tile([128, TOKPC], F16, tag="H", bufs=3)
                    nc.gpsimd.tensor_scalar(H[:], sg1[:], 0.5, 0.5,
                                            OP.mult, OP.add)
                    # encode pipeline: prepare one-hot planes ahead (layer 0)
                    if layer == 0:
                        if t + 4 < T:
                            make_ge(t + 4)
                        if t + 3 < T:
                            make_onehot(t + 3)
                    # ---- previous step's output-side work (keeps PE fed) ----
                    if prev is not None:
                        emit_mm2_lif2(*prev, layer)
                        if layer == L - 1:
                            flush_tips(keep=4)
                    prev = (t, H, xs)
                    Hprev = H
                emit_mm2_lif2(*prev, layer)
                flush_tips()

            # time-integrated rates -> bf16 (ACT, scale = 1/T)
            for k in range(KC):
                nc.scalar.activation(tibf[:, k * 256:(k + 1) * 256],
                                     tips[:, k * TOKPC:(k + 1) * TOKPC],
                                     ACTF.Copy, bias=0.0, scale=1.0 / T)

        # ---- full-vocab projection of this core's own 256 tokens ----------
        # (no collective: Wp is streamed in full, out is token-sharded)
        with tc.tile_pool(name="prjp", bufs=8, space="PSUM") as prjps, \
             tc.tile_pool(name="osb", bufs=10) as osbp:
            for sb in range(NSB):
                if sb + PREF < NSB:
                    fetch_wpb(sb + PREF)
                wpb = wpb_tiles[sb]
                for j in range(SBJ):
                    vb = sb * SBJ + j
                    po = prjps.tile([VB, TOKPC], F32, tag="po", name="po",
                                    bufs=8)
                    for k in range(KC):
                        nc.tensor.matmul(po[:],
                                         wpb[:, k, j * VB:(j + 1) * VB],
                                         tibf[:, k * 256:(k + 1) * 256],
                                         start=(k == 0), stop=(k == KC - 1),
                                         skip_group_check=True)
                    osb = osbp.tile([VB, TOKPC], BF16, tag="osb", bufs=10)
                    if vb % 2 == 0:
                        nc.vector.tensor_scalar(osb[:], po[:],
                                                biasb[:, vb:vb + 1], None, OP.add)
                    else:
                        nc.scalar.activation(osb[:], po[:], ACTF.Identity,
                                             bias=biasb[:, vb:vb + 1], scale=1.0)
                    eng = nc.sync if vb % 2 == 0 else nc.gpsimd
                    eng.dma_start(out_d[vb * VB:(vb + 1) * VB, :], osb[:])

    nc.compile()
    return nc


_NC_CACHE = {}
_last_in_maps = None


def _get_nc():
    if "nc" not in _NC_CACHE:
        _NC_CACHE["nc"] = _build_nc()
    return _NC_CACHE["nc"]


def kernel(input_ids, emb_table, A, B, C, D, Wp, bp):
    input_ids = np.asarray(input_ids)
    emb_table = np.ascontiguousarray(np.asarray(emb_table), dtype=np.float32)
    A = np.asarray(A, dtype=np.float32)
    B = np.asarray(B, dtype=np.float32)
    C = np.asarray(C, dtype=np.float32)
    D = np.asarray(D, dtype=np.float32)
    Wp = np.asarray(Wp, dtype=np.float32)
    bp = np.asarray(bp, dtype=np.float32)

    ids_flat = input_ids.reshape(-1).astype(np.int32)          # (2048,)

    at = np.ascontiguousarray(A.transpose(0, 2, 1))            # (L,128,128)
    at_hi, at_lo = _hilo16(at)
    bt = np.ascontiguousarray(
        B.transpose(2, 0, 1).reshape(KC, 128, L, DS).transpose(2, 1, 0, 3))
    # bt[l,p,k,m] = B[l, m, k*128+p]
    bt_hi, bt_lo = _hilo16(bt)
    ct = np.ascontiguousarray(C.transpose(0, 2, 1).reshape(L, 128, KC, 128))
    # ct[l,p,mc,m] = C[l, mc*128+m, p]
    ct_hi, ct_lo = _hilo16(ct)
    # dc[p, l*KC+k] = D[l, k*128+p]; fp16-rounded values shipped as f32
    dc = np.ascontiguousarray(
        D.reshape(L, KC, 128).transpose(2, 0, 1).reshape(128, L * KC))
    dc_hi16, dc_lo16 = _hilo16(dc)
    dc_hi = dc_hi16.astype(np.float32)
    dc_lo = dc_lo16.astype(np.float32)

    wpt = np.ascontiguousarray(Wp.T)                           # (512, 32000) f32
    wpt_bf = wpt.astype(ml_dtypes.bfloat16)
    # wptb[sb, p, k, v] = Wp.T[k*128+p, sb*640+v]  (full vocab, shared)
    wptb = np.ascontiguousarray(
        wpt_bf.reshape(KC, 128, NSB, SBV).transpose(2, 1, 0, 3))
    bsh = np.ascontiguousarray(bp.reshape(NVBF, VB).T)         # (128, 250)

    nc = _get_nc()
    in_maps = []
    for c in range(NCORES):
        ids_c = ids_flat[c * TOKPC:(c + 1) * TOKPC].reshape(2, 128, 1)
        in_maps.append({
            "ids": np.ascontiguousarray(ids_c),
            "emb": emb_table,
            "at_hi": at_hi, "at_lo": at_lo,
            "bt_hi": bt_hi, "bt_lo": bt_lo,
            "ct_hi": ct_hi, "ct_lo": ct_lo,
            "dc_hi": dc_hi, "dc_lo": dc_lo,
            "wptb": wptb,
            "bias": bsh,
        })

    global _last_in_maps
    _last_in_maps = in_maps
    res = run_bass_kernel_spmd(nc, in_maps, core_ids=list(range(NCORES)))
    outs = [np.asarray(res.results[c]["out"]) for c in range(NCORES)]
    full = np.concatenate(outs, axis=1)                        # (32000, 2048) bf16
    full = full.astype(np.float32).T                           # (2048, 32000)
    return np.ascontiguousarray(full).reshape(BATCH, SEQ, VOC)


# revision 40
# speedup vs baseline: 1.1453x; 1.0504x over previous
"""Trainium2 Bass kernel for nn_BreakthroughSNN (spiking SSM LM).

v3 strategy (8 NeuronCores, SPMD single NEFF):
  - Data-parallel SSM: 2048 tokens (B*S) sharded 256/core; 4 layers x 20
    steps of LIF recurrence with membrane potentials resident in PSUM.
  - All SSM matmul operands in fp16: weights as host-split fp16 hi/lo
    pairs (11+11 mantissa bits ~ exact fp32 grade), spikes are {0,1}
    (exact in fp16).  Halves LDWEIGHTS traffic vs fp32r and avoids the
    fp32r producer-rounding rule.
  - Temporal encoding pipelined under layer 0: cumulative ge-planes
    (emb >= T_t) on DVE, one-hot diff on Pool, no separate encode phase.
  - LIF work spread so no engine exceeds the PE: ACT does Sign/masks,
    Pool does spike remaps, DVE only the PSUM read-modify-writes.
  - tips (time integration) matmuls emitted with 2-step skew so the PE
    never waits on the spike-write chain.
  - AllGather of bf16 rates; vocab-sharded projection transposed (vocab
    on PSUM partitions, per-partition bias, bf16 output).
"""

import numpy as np
import ml_dtypes
from contextlib import ExitStack

import concourse.bass as bass
import concourse.mybir as mybir
import concourse.tile as tile
from concourse import bacc
from concourse.bass_utils import run_bass_kernel_spmd
from concourse.masks import make_identity

F32 = mybir.dt.float32
F16 = mybir.dt.float16
BF16 = mybir.dt.bfloat16
I32 = mybir.dt.int32
OP = mybir.AluOpType
ACTF = mybir.ActivationFunctionType

NCORES = 8
TOKPC = 256          # tokens per core
BATCH, SEQ = 4, 512
DM, DS = 512, 128
T, L = 20, 4
VOC = 32000
VSH = VOC // NCORES  # 4000 vocab per core
KC = DM // 128       # 4 feature chunks
VB = 128             # vocab rows per projection block (full PE width)
NVBF = VOC // VB     # 250 blocks over the FULL vocab (per core)
SBJ = 5              # blocks per streamed super-block
SBV = VB * SBJ       # 640 vocab rows per super-block
NSB = NVBF // SBJ    # 50 streamed super-blocks
PREF = 16            # super-blocks prefetched during the SSM phase


def _hilo16(x):
    x = np.ascontiguousarray(x, dtype=np.float32)
    hi = x.astype(np.float16)
    lo = (x - hi.astype(np.float32)).astype(np.float16)
    return hi, lo


def _f2key(x):
    u = int(np.array(x, dtype=np.float32).view(np.uint32))
    return (u ^ 0x80000000) if u < 0x80000000 else (0xFFFFFFFF - u)


def _key2f(k):
    u = (k ^ 0x80000000) if k >= 0x80000000 else (0xFFFFFFFF - k)
    return np.array([u], dtype=np.uint32).view(np.float32)[0]


def _g32(x):
    # replicate reference fp32 pipeline: floor happens on this value
    x = np.float32(x)
    s = np.float32(1.0) / (np.float32(1.0) + np.float32(np.exp(np.float32(-x))))
    return np.float32(s * np.float32(19.0))


def _thresholds():
    """T_k = smallest fp32 x with g32(x) >= k, k=1..19 (g32 monotone)."""
    ts = []
    for k in range(1, 20):
        lo_k = _f2key(np.float32(-30.0))
        hi_k = _f2key(np.float32(30.0))
        assert _g32(_key2f(hi_k)) >= k and _g32(_key2f(lo_k)) < k
        while hi_k - lo_k > 1:
            mid = (lo_k + hi_k) // 2
            if _g32(_key2f(mid)) >= k:
                hi_k = mid
            else:
                lo_k = mid
        ts.append(float(_key2f(hi_k)))
    return ts


def _build_nc():
    nc = bacc.Bacc("TRN2", target_bir_lowering=False, debug=False, num_devices=NCORES)

    ids_d = nc.dram_tensor("ids", [2, 128, 1], I32, kind="ExternalInput")
    emb_d = nc.dram_tensor("emb", [VOC, DM], F32, kind="ExternalInput")
    at_hi_d = nc.dram_tensor("at_hi", [L, 128, 128], F16, kind="ExternalInput")
    at_lo_d = nc.dram_tensor("at_lo", [L, 128, 128], F16, kind="ExternalInput")
    bt_hi_d = nc.dram_tensor("bt_hi", [L, 128, KC, 128], F16, kind="ExternalInput")
    bt_lo_d = nc.dram_tensor("bt_lo", [L, 128, KC, 128], F16, kind="ExternalInput")
    ct_hi_d = nc.dram_tensor("ct_hi", [L, 128, KC, 128], F16, kind="ExternalInput")
    ct_lo_d = nc.dram_tensor("ct_lo", [L, 128, KC, 128], F16, kind="ExternalInput")
    # D hi/lo values (fp16-representable) stored as f32 for the scalar AP
    dc_hi_d = nc.dram_tensor("dc_hi", [128, L * KC], F32, kind="ExternalInput")
    dc_lo_d = nc.dram_tensor("dc_lo", [128, L * KC], F32, kind="ExternalInput")
    wptb_d = nc.dram_tensor("wptb", [NSB, 128, KC, SBV], BF16, kind="ExternalInput")
    bias_d = nc.dram_tensor("bias", [VB, NVBF], F32, kind="ExternalInput")
    out_d = nc.dram_tensor("out", [VOC, TOKPC], BF16, kind="ExternalOutput")

    THR = _thresholds()

    with tile.TileContext(nc) as tc, ExitStack() as ctx:
        const = ctx.enter_context(tc.tile_pool(name="const", bufs=1))
        ident = const.tile([128, 128], F32)
        make_identity(nc, ident[:])
        ident16 = const.tile([128, 128], F16)
        nc.gpsimd.tensor_scalar(ident16[:], ident[:], 1.0, 0.0, OP.mult, OP.add)
        neg2 = const.tile([128, 1], F32)
        nc.vector.memset(neg2[:], -2.0)

        # ---- persistent SBUF state ----
        xb_pool = ctx.enter_context(tc.tile_pool(name="xb", bufs=1))
        xb = xb_pool.tile([128, T * KC * 256], F16)
        tip = ctx.enter_context(tc.tile_pool(name="ti", bufs=1))
        tibf = tip.tile([128, KC * 256], BF16, tag="tibf")
        embp = ctx.enter_context(tc.tile_pool(name="embp", bufs=1))
        EMBall = embp.tile([128, KC * 256], F32, tag="emba")
        gep = ctx.enter_context(tc.tile_pool(name="gep", bufs=3))

        # ---- encode inputs first: gather + transpose (short critical path) --
        with tc.tile_pool(name="enc", bufs=2) as enc, \
             tc.tile_pool(name="encp", bufs=2, space="PSUM") as encps:
            ids_s = enc.tile([128, 2], I32, tag="ids")
            for g in range(2):
                nc.sync.dma_start(ids_s[:, g:g + 1], ids_d[g, :, :])
            for g in range(2):
                eg = enc.tile([128, DM], F32, tag="eg")
                nc.gpsimd.indirect_dma_start(
                    out=eg[:], out_offset=None,
                    in_=emb_d[:, :],
                    in_offset=bass.IndirectOffsetOnAxis(ap=ids_s[:, g:g + 1], axis=0),
                )
                for k in range(KC):
                    pt = encps.tile([128, 128], F32, tag="pt")
                    nc.tensor.transpose(pt[:], eg[:, k * 128:(k + 1) * 128], ident[:])
                    nc.scalar.copy(EMBall[:, k * 256 + g * 128:k * 256 + g * 128 + 128],
                                   pt[:])

        # ge-plane pipeline: ge[t] = (emb >= T_t), xb[t] = ge[t] - ge[t+1]
        ges = {}

        def make_ge(t):
            # t in 1..19; ge_20 == 0, ge_0 == 1  ({0,1} exact in fp16)
            g_ = gep.tile([128, KC * 256], F16, tag="ge", name=f"ge{t}", bufs=4)
            nc.vector.tensor_scalar(g_[:], EMBall[:], float(THR[t - 1]), None,
                                    OP.is_ge)
            ges[t] = g_

        def make_onehot(t):
            # diff into fp16 xb plane; alternate DVE/Pool (Pool TT is slow,
            # DVE gets 2x throughput on 16-bit)
            dst = xb[:, t * KC * 256:(t + 1) * KC * 256]
            eng = nc.vector if t % 2 == 0 else nc.gpsimd
            if t == 0:
                # 1 - ge_1
                eng.tensor_scalar(dst, ges[1][:], -1.0, 1.0, OP.mult, OP.add)
            elif t == T - 1:
                eng.tensor_scalar(dst, ges[T - 1][:], 1.0, 0.0, OP.mult, OP.add)
            else:
                eng.tensor_tensor(dst, ges[t][:], ges[t + 1][:], OP.subtract)

        make_ge(1)
        make_onehot(0)
        make_ge(2)
        make_onehot(1)
        make_ge(3)
        make_onehot(2)

        # ---- all-layer parameters (fp16, plain DMA) ----
        par = ctx.enter_context(tc.tile_pool(name="par", bufs=1))
        AH, AL, BH, BL, CH, CL = [], [], [], [], [], []
        for l in range(L):
            for nm, lst, dram, shp in (
                    ("ah", AH, at_hi_d, (128, 128)), ("al", AL, at_lo_d, (128, 128)),
                    ("bh", BH, bt_hi_d, (128, KC, 128)), ("bl", BL, bt_lo_d, (128, KC, 128)),
                    ("ch", CH, ct_hi_d, (128, KC, 128)), ("cl", CL, ct_lo_d, (128, KC, 128))):
                t_ = par.tile(list(shp), F16, tag=f"p_{nm}_{l}", name=f"par{l}{nm}")
                nc.sync.dma_start(t_[:], dram[l])
                lst.append(t_)
        dch_all = par.tile([128, L * KC], F32, tag="dch")
        nc.sync.dma_start(dch_all[:], dc_hi_d[:, :])
        dcl_all = par.tile([128, L * KC], F32, tag="dcl")
        nc.sync.dma_start(dcl_all[:], dc_lo_d[:, :])

        # ---- projection weights: streamed super-blocks; prefetch the first
        # PREF during the (HBM-idle) SSM phase via the Pool DMA queue ----
        wpp = ctx.enter_context(tc.tile_pool(name="wpp", bufs=PREF))
        wpb_tiles = {}

        def fetch_wpb(sb):
            w = wpp.tile([128, KC, SBV], BF16, tag="wpb", name=f"wpb{sb}",
                         bufs=PREF)
            # upfront prefetch rides the sync queue (idle during SSM); the
            # in-projection stream rides the Pool queue (idle then) so
            # neither blocks behind output DMAs
            eng = nc.sync if sb < PREF else nc.gpsimd
            eng.dma_start(w[:], wptb_d[sb])
            wpb_tiles[sb] = w

        biasp = ctx.enter_context(tc.tile_pool(name="biasp", bufs=1))
        biasb = biasp.tile([VB, NVBF], F32, tag="biasb")
        nc.sync.dma_start(biasb[:], bias_d[:, :])
        for sb in range(PREF):
            fetch_wpb(sb)

        # ---------------- SSM layers ---------------------------------------
        with tc.tile_pool(name="ssmp", bufs=1, space="PSUM") as ssmps, \
             tc.tile_pool(name="dgp", bufs=2) as dgp, \
             tc.tile_pool(name="lif", bufs=3) as lif:
            v1ps = ssmps.tile([128, TOKPC], F32, tag="v1")
            v2pr = [ssmps.tile([128, 2 * TOKPC], F32, tag=f"v2p{j}", name=f"v2pr{j}")
                    for j in range(2)]
            tips = ssmps.tile([128, KC * TOKPC], F32, tag="tips")

            Hprev = None
            tips_pend = []  # deferred tips matmuls (2-step skew)
            for layer in range(L):
                # diagonal D tiles for this layer, built on Pool (fast path)
                ddh, ddl = [], []
                for k in range(KC):
                    dt_ = dgp.tile([128, 128], F16, tag=f"ddh{k}", name=f"ddh{k}")
                    nc.gpsimd.tensor_scalar(
                        dt_[:], ident[:],
                        dch_all[:, layer * KC + k:layer * KC + k + 1], 0.0,
                        OP.mult, OP.add)
                    ddh.append(dt_)
                    dt_ = dgp.tile([128, 128], F16, tag=f"ddl{k}", name=f"ddl{k}")
                    nc.gpsimd.tensor_scalar(
                        dt_[:], ident[:],
                        dcl_all[:, layer * KC + k:layer * KC + k + 1], 0.0,
                        OP.mult, OP.add)
                    ddl.append(dt_)
                ahT, alT = AH[layer], AL[layer]
                bhT, blT = BH[layer], BL[layer]
                chT, clT = CH[layer], CL[layer]

                def flush_tips(keep=0):
                    while len(tips_pend) > keep:
                        t_, k_, xs_ = tips_pend.pop(0)
                        nc.tensor.matmul(
                            tips[:, k_ * TOKPC:(k_ + 1) * TOKPC],
                            ident16[:], xs_,
                            start=(t_ == 0 and k_ % 2 == 0),
                            stop=(t_ == T - 1),
                            skip_group_check=True)

                def emit_mm2_lif2(t, H_t, xs_t, layer_):
                    # output update accumulation (v2, per chunk) + LIF2
                    for k in range(KC):
                        vsl = v2pr[k // 2][:, (k % 2) * TOKPC:(k % 2 + 1) * TOKPC]
                        mm2 = [(chT[:, k, :], H_t[:]), (clT[:, k, :], H_t[:]),
                               (ddh[k][:], xs_t[k]), (ddl[k][:], xs_t[k])]
                        for i, (lhsT, rhs) in enumerate(mm2):
                            nc.tensor.matmul(vsl, lhsT, rhs,
                                             start=(t == 0 and i == 0 and k % 2 == 0),
                                             stop=(i == len(mm2) - 1),
                                             skip_group_check=True)
                    # j=0: DVE mask from PSUM, Pool spike remap
                    m2 = lif.tile([128, 2 * TOKPC], F32, tag="m2_0", name="m2_0")
                    nc.vector.tensor_scalar(m2[:], v2pr[0][:], 2.0, 0.5,
                                            OP.is_lt, OP.mult)
                    nc.vector.tensor_tensor(v2pr[0][:], v2pr[0][:], m2[:], OP.mult)
                    xsl0 = xb[:, t * KC * 256:t * KC * 256 + 512]
                    nc.gpsimd.tensor_scalar(xsl0, m2[:], -2.0, 1.0, OP.mult, OP.add)
                    # j=1: ACT sign + mask, Pool spike remap
                    sg2 = lif.tile([128, 2 * TOKPC], F32, tag="sg2_1", name="sg2_1")
                    nc.scalar.activation(sg2[:], v2pr[1][:], ACTF.Sign,
                                         bias=neg2[:], scale=1.0)
                    m2b = lif.tile([128, 2 * TOKPC], F32, tag="m2_1", name="m2_1")
                    nc.scalar.activation(m2b[:], sg2[:], ACTF.Copy,
                                         bias=0.25, scale=-0.25)
                    nc.vector.tensor_tensor(v2pr[1][:], v2pr[1][:], m2b[:], OP.mult)
                    xsl1 = xb[:, t * KC * 256 + 512:t * KC * 256 + 1024]
                    nc.gpsimd.tensor_scalar(xsl1, sg2[:], 0.5, 0.5, OP.mult, OP.add)
                    if layer_ == L - 1:
                        for k in range(KC):
                            tips_pend.append((t, k, xs_t[k]))

                prev = None  # (t, H, xs) pending MM2+LIF2 (1-step software skew)
                for t in range(T):
                    xs = [xb[:, (t * KC + k) * 256:(t * KC + k) * 256 + 256]
                          for k in range(KC)]
                    # ---- state update accumulation (v1): B first, A last ----
                    mm1 = []
                    for k in range(KC):
                        mm1 += [(bhT[:, k, :], xs[k]), (blT[:, k, :], xs[k])]
                    if t > 0:
                        mm1 += [(ahT[:], Hprev[:]), (alT[:], Hprev[:])]
                    for i, (lhsT, rhs) in enumerate(mm1):
                        nc.tensor.matmul(v1ps[:], lhsT, rhs,
                                         start=(t == 0 and i == 0),
                                         stop=(i == len(mm1) - 1),
                                         skip_group_check=True)
                    # ---- LIF1: ACT sign, Pool masks, DVE RMW only ----
                    sg1 = lif.tile([128, TOKPC], F32, tag="sg1")
                    nc.scalar.activation(sg1[:], v1ps[:], ACTF.Sign,
                                         bias=neg2[:], scale=1.0)
                    m1 = lif.tile([128, TOKPC], F32, tag="m1")
                    nc.gpsimd.tensor_scalar(m1[:], sg1[:], -0.25, 0.25,
                                            OP.mult, OP.add)
                    nc.vector.tensor_tensor(v1ps[:], v1ps[:], m1[:], OP.mult)
                    H = lif.tile([128, TOKPC], F16, tag="H", bufs=3)
                    nc.gpsimd.tensor_scalar(H[:], sg1[:], 0.5, 0.5,
                                            OP.mult, OP.add)
                    # encode pipeline: prepare one-hot planes ahead (layer 0)
                    if layer == 0:
                        if t + 4 < T:
                            make_ge(t + 4)
                        if t + 3 < T:
                            make_onehot(t + 3)
                    # ---- previous step's output-side work (keeps PE fed) ----
                    if prev is not None:
                        emit_mm2_lif2(*prev, layer)
                        if layer == L - 1:
                            flush_tips(keep=4)
                    prev = (t, H, xs)
                    Hprev = H
                emit_mm2_lif2(*prev, layer)
                flush_tips()

            # time-integrated rates -> bf16 (ACT, scale = 1/T)
            for k in range(KC):
                nc.scalar.activation(tibf[:, k * 256:(k + 1) * 256],
                                     tips[:, k * TOKPC:(k + 1) * TOKPC],
                                     ACTF.Copy, bias=0.0, scale=1.0 / T)

        # ---- full-vocab projection of this core's own 256 tokens ----------
        # (no collective: Wp is streamed in full, out is token-sharded)
        with tc.tile_pool(name="prjp", bufs=8, space="PSUM") as prjps, \
             tc.tile_pool(name="osb", bufs=4) as osbp:
            for sb in range(NSB):
                if sb + PREF < NSB:
                    fetch_wpb(sb + PREF)
                wpb = wpb_tiles[sb]
                # one coalesced output tile + DMA per super-block (5 blocks)
                osb5 = osbp.tile([VB, SBJ * TOKPC], BF16, tag="osb5",
                                 name="osb5", bufs=4)
                for j in range(SBJ):
                    vb = sb * SBJ + j
                    po = prjps.tile([VB, TOKPC], F32, tag="po", name="po",
                                    bufs=8)
                    for k in range(KC):
                        nc.tensor.matmul(po[:],
                                         wpb[:, k, j * VB:(j + 1) * VB],
                                         tibf[:, k * 256:(k + 1) * 256],
                                         start=(k == 0), stop=(k == KC - 1),
                                         skip_group_check=True)
                    oslice = osb5[:, j * TOKPC:(j + 1) * TOKPC]
                    if vb % 2 == 0:
                        nc.vector.tensor_scalar(oslice, po[:],
                                                biasb[:, vb:vb + 1], None, OP.add)
                    else:
                        nc.scalar.activation(oslice, po[:], ACTF.Identity,
                                             bias=biasb[:, vb:vb + 1], scale=1.0)
                # dst iterated (p, j, tok): row = j*128 + p within the
                # 640-row super-block
                dsl = out_d[sb * SBV:(sb + 1) * SBV, :]
                dst = bass.AP(tensor=dsl.tensor, offset=dsl.offset,
                              ap=[[TOKPC, VB], [VB * TOKPC, SBJ], [1, TOKPC]])
                eng = nc.sync if sb % 2 == 0 else nc.gpsimd
                eng.dma_start(dst, osb5[:])

    nc.compile()
    return nc


_NC_CACHE = {}
_last_in_maps = None


def _get_nc():
    if "nc" not in _NC_CACHE:
        _NC_CACHE["nc"] = _build_nc()
    return _NC_CACHE["nc"]


def kernel(input_ids, emb_table, A, B, C, D, Wp, bp):
    input_ids = np.asarray(input_ids)
    emb_table = np.ascontiguousarray(np.asarray(emb_table), dtype=np.float32)
    A = np.asarray(A, dtype=np.float32)
    B = np.asarray(B, dtype=np.float32)
    C = np.asarray(C, dtype=np.float32)
    D = np.asarray(D, dtype=np.float32)
    Wp = np.asarray(Wp, dtype=np.float32)
    bp = np.asarray(bp, dtype=np.float32)

    ids_flat = input_ids.reshape(-1).astype(np.int32)          # (2048,)

    at = np.ascontiguousarray(A.transpose(0, 2, 1))            # (L,128,128)
    at_hi, at_lo = _hilo16(at)
    bt = np.ascontiguousarray(
        B.transpose(2, 0, 1).reshape(KC, 128, L, DS).transpose(2, 1, 0, 3))
    # bt[l,p,k,m] = B[l, m, k*128+p]
    bt_hi, bt_lo = _hilo16(bt)
    ct = np.ascontiguousarray(C.transpose(0, 2, 1).reshape(L, 128, KC, 128))
    # ct[l,p,mc,m] = C[l, mc*128+m, p]
    ct_hi, ct_lo = _hilo16(ct)
    # dc[p, l*KC+k] = D[l, k*128+p]; fp16-rounded values shipped as f32
    dc = np.ascontiguousarray(
        D.reshape(L, KC, 128).transpose(2, 0, 1).reshape(128, L * KC))
    dc_hi16, dc_lo16 = _hilo16(dc)
    dc_hi = dc_hi16.astype(np.float32)
    dc_lo = dc_lo16.astype(np.float32)

    wpt = np.ascontiguousarray(Wp.T)                           # (512, 32000) f32
    wpt_bf = wpt.astype(ml_dtypes.bfloat16)
    # wptb[sb, p, k, v] = Wp.T[k*128+p, sb*640+v]  (full vocab, shared)
    wptb = np.ascontiguousarray(
        wpt_bf.reshape(KC, 128, NSB, SBV).transpose(2, 1, 0, 3))
    bsh = np.ascontiguousarray(bp.reshape(NVBF, VB).T)         # (128, 250)

    nc = _get_nc()
    in_maps = []
    for c in range(NCORES):
        ids_c = ids_flat[c * TOKPC:(c + 1) * TOKPC].reshape(2, 128, 1)
        in_maps.append({
            "ids": np.ascontiguousarray(ids_c),
            "emb": emb_table,
            "at_hi": at_hi, "at_lo": at_lo,
            "bt_hi": bt_hi, "bt_lo": bt_lo,
            "ct_hi": ct_hi, "ct_lo": ct_lo,
            "dc_hi": dc_hi, "dc_lo": dc_lo,
            "wptb": wptb,
            "bias": bsh,
        })

    global _last_in_maps
    _last_in_maps = in_maps
    res = run_bass_kernel_spmd(nc, in_maps, core_ids=list(range(NCORES)))
    outs = [np.asarray(res.results[c]["out"]) for c in range(NCORES)]
    full = np.concatenate(outs, axis=1)                        # (32000, 2048) bf16
    full = full.astype(np.float32).T                           # (2048, 32000)
    return np.ascontiguousarray(full).reshape(BATCH, SEQ, VOC)
